# revision 10
# baseline (speedup 1.0000x reference)
"""Trainium2 Bass kernel for nn_AC_Filter_PreNorm_Net (causal attention + product-network Euler).

Self-contained: accepts FULL inputs, shards batch over 8 NeuronCores, returns FULL output.

Restructured dataflow (validated against reference in numpy, rel err ~6e-3 all-bf16):
  - sigma pre-norm folded into in_proj weights (host)
  - transposed activations: qT/kT [e, L] so scores come out as sT[kv, q]
  - no-max-subtraction softmax (max |score| ~ 13, exp fine in fp32)
  - v augmented with a ones column -> attention output row 64 = softmax denominator
  - out_proj matmul carries the denominator (Wout_aug row-0 selector)
  - normalization deferred until after out_proj, applied once via a DRAM-broadcast
    reciprocal row
  - Euler product tree with host-reordered Wall columns so every level pairs
    (k, k+half) contiguously; "transposes" done as normal matmuls against identity
"""
import sys
sys.path.insert(0, "/opt/trn_rl_repo")
import numpy as np
import concourse.bass as bass
import concourse.tile as tile
import bass_rust
from concourse import mybir
from concourse.bass_utils import run_bass_kernel_spmd

F32 = mybir.dt.float32
BF16 = mybir.dt.bfloat16
AF = mybir.ActivationFunctionType
MULT = mybir.AluOpType.mult
ADD = mybir.AluOpType.add

B, L, D = 16, 2048, 63
E = D + 1            # 64
W1 = 8
F_LEN = 4
DT = 0.01
EPS = 1e-5
NCORES = 8
BPC = B // NCORES    # batches per core = 2
NT = L // 128        # l-tiles per batch = 16
NC4 = 4              # q-chunks of 512


def _split_multiwaits(nc):
    """walrus here rejects >1 sync wait per instruction; hoist extras onto
    preceding same-engine NoOps."""
    n_added = 0
    for fn in nc.m.functions:
        for bb in fn.blocks:
            insts = list(bb.instructions)
            out = []
            changed = False
            for inst in insts:
                si = inst.sync_info
                if si is not None and si.on_wait is not None and len(si.on_wait) > 1:
                    waits = list(si.on_wait)
                    for w in waits[:-1]:
                        nop = mybir.InstNoOp(
                            name=f"{inst.name}-wsp{n_added}", ins=[], outs=[]
                        )
                        n_added += 1
                        nop.engine = inst.engine
                        nop.sync_info = bass_rust.SyncInfo(on_wait=[w], on_update=[])
                        out.append(nop)
                    si.on_wait = [waits[-1]]
                    changed = True
                out.append(inst)
            if changed:
                bb.instructions = out
    return n_added


def _build_nc():
    nc = bass.Bass()
    dp = nc.declare_dram_parameter
    xt_e = dp("xt", [BPC, E, L], BF16, isOutput=False)          # host-pretransposed
    wqkt_e = dp("wqkt", [E, 128], BF16, isOutput=False)         # lhsT: [e_in, q|k out]
    wvt_e = dp("wvt", [E, E], BF16, isOutput=False)             # rhs: [e_in, e_out]
    woutkt_e = dp("woutkt", [E + 1, E], BF16, isOutput=False)   # [65, 64] both lhsT & rhs
    wall_e = dp("wall", [E, D * W1], BF16, isOutput=False)      # [64, 504] tree-ordered
    masks_e = dp("masks", [128, 4 * 512], BF16, isOutput=False)
    ident_e = dp("ident", [128, 128], BF16, isOutput=False)
    srep_e = dp("srep", [128, E], F32, isOutput=False)          # col0=0, col 1+d = s[d]
    out_e = dp("out", [BPC, L, F_LEN * D], F32, isOutput=True)
    denrow_d = nc.dram_tensor("denrow", [BPC, L], BF16)
    rdenrow_d = nc.dram_tensor("rdenrow", [BPC, L], F32)

    with tile.TileContext(nc) as tc:
        with (
            tc.tile_pool(name="consts", bufs=1) as cp,
            tc.tile_pool(name="big", bufs=2) as bp,
            tc.tile_pool(name="small", bufs=2) as sp,
            tc.tile_pool(name="outp", bufs=3) as op_pool,
            tc.tile_pool(name="ps", bufs=4, space="PSUM") as psP,
        ):
            # ---- constants ----
            wqkt = cp.tile([E, 128], BF16)
            nc.sync.dma_start(out=wqkt[:], in_=wqkt_e[:])
            wvt = cp.tile([E, E], BF16)
            nc.sync.dma_start(out=wvt[:], in_=wvt_e[:])
            woutkt = cp.tile([E + 1, E], BF16)
            nc.sync.dma_start(out=woutkt[:], in_=woutkt_e[:])
            wall = cp.tile([E, D * W1], BF16)
            nc.sync.dma_start(out=wall[:], in_=wall_e[:])
            masks = cp.tile([128, 4 * 512], BF16)
            nc.sync.dma_start(out=masks[:], in_=masks_e[:])
            ident = cp.tile([128, 128], BF16)
            nc.sync.dma_start(out=ident[:], in_=ident_e[:])
            srep = cp.tile([128, E], F32)
            nc.sync.dma_start(out=srep[:], in_=srep_e[:])

            for b in range(BPC):
                # ---- phase A: load xT, project q/k/v ----
                xt = bp.tile([E, L], BF16, tag="xt")
                nc.sync.dma_start(out=xt[:], in_=xt_e[b])

                qT = bp.tile([E, L], BF16, tag="qT")
                kT = bp.tile([E, L], BF16, tag="kT")
                for cp in range(2):
                    ps = psP.tile([128, 1024], F32, tag="ps")
                    for u in range(2):
                        c = 2 * cp + u
                        nc.tensor.matmul(
                            ps[:, u * 512:(u + 1) * 512], wqkt[:],
                            xt[:, c * 512:(c + 1) * 512],
                            start=True, stop=True,
                        )
                    nc.scalar.copy(qT[:, cp * 1024:(cp + 1) * 1024], ps[0:E, :])
                    nc.vector.tensor_copy(kT[:, cp * 1024:(cp + 1) * 1024],
                                          ps[64:128, :])

                # v in [l, e] tiles + ones column (65-stride layout)
                v_aug = bp.tile([128, NT * (E + 1)], BF16, tag="v_aug")
                v_aug_v = v_aug[:].rearrange("p (n e1) -> p n e1", e1=E + 1)
                nc.vector.memset(v_aug_v[:, :, E:E + 1], 1.0)
                ps = psP.tile([128, 1024], F32, tag="ps")
                for lt in range(NT):
                    nc.tensor.matmul(
                        ps[:, lt * E:(lt + 1) * E],
                        xt[:, lt * 128:(lt + 1) * 128], wvt[:],
                        start=True, stop=True,
                    )
                nc.vector.tensor_copy(
                    v_aug_v[:, :, 0:E],
                    ps[:].rearrange("p (j e) -> p j e", e=E),
                )

                # ---- phase C: attention ----
                exps = bp.tile([128, NT * 512], BF16, tag="exps")
                o_un = bp.tile([E + 1, L], BF16, tag="o_un")
                for c in range(NC4):
                    nki = 4 * c + 4
                    ki = 0
                    while ki < nki:
                        g = min(2, nki - ki)
                        ps = psP.tile([128, 1024], F32, tag="ps")
                        for j in range(g):
                            nc.tensor.matmul(
                                ps[:, j * 512:(j + 1) * 512],
                                kT[:, (ki + j) * 128:(ki + j + 1) * 128],
                                qT[:, c * 512:(c + 1) * 512],
                                start=True, stop=True,
                            )
                        nc.scalar.activation(
                            exps[:, ki * 512:(ki + g) * 512],
                            ps[:, 0:g * 512], AF.Exp,
                        )
                        ki += g
                    # causal masks on the 4 diagonal-region blocks (gpsimd)
                    for off in range(4):
                        kb = 4 * c + off
                        nc.gpsimd.tensor_tensor(
                            exps[:, kb * 512:(kb + 1) * 512],
                            exps[:, kb * 512:(kb + 1) * 512],
                            masks[:, off * 512:(off + 1) * 512],
                            MULT,
                        )
                    # attn @ [v|1] accumulated over ki
                    if c % 2 == 0:
                        po = psP.tile([128, 1024], F32, tag="ps")
                    pov = po[0:E + 1, (c % 2) * 512:((c % 2) + 1) * 512]
                    for ki in range(nki):
                        nc.tensor.matmul(
                            pov,
                            v_aug[:, ki * (E + 1):(ki + 1) * (E + 1)],
                            exps[:, ki * 512:(ki + 1) * 512],
                            start=(ki == 0), stop=(ki == nki - 1),
                        )
                    nc.scalar.copy(o_un[:, c * 512:(c + 1) * 512], pov)
                    # denominator row -> DRAM (bf16)
                    nc.sync.dma_start(
                        out=denrow_d[b, c * 512:(c + 1) * 512],
                        in_=o_un[E:E + 1, c * 512:(c + 1) * 512],
                    )

                # out_proj (unnormalized stateT, row0 = denom)
                stu = bp.tile([E, L], BF16, tag="stu")
                for cp in range(2):
                    ps = psP.tile([128, 1024], F32, tag="ps")
                    for u in range(2):
                        c = 2 * cp + u
                        nc.tensor.matmul(
                            ps[0:E, u * 512:(u + 1) * 512], woutkt[:],
                            o_un[:, c * 512:(c + 1) * 512],
                            start=True, stop=True,
                        )
                    nc.scalar.copy(stu[:, cp * 1024:(cp + 1) * 1024], ps[0:E, :])

                # ---- phase D: reciprocal roundtrip + normalize ----
                den_sp = sp.tile([128, NT], BF16, tag="den_sp")
                nc.sync.dma_start(
                    out=den_sp[:],
                    in_=bass.AP(tensor=denrow_d, offset=b * L,
                                ap=[[1, 128], [128, NT]]),
                )
                rden_sp = sp.tile([128, NT], F32, tag="rden_sp")
                nc.vector.reciprocal(rden_sp[:], den_sp[:])
                nc.sync.dma_start(
                    out=bass.AP(tensor=rdenrow_d, offset=b * L,
                                ap=[[1, 128], [128, NT]]),
                    in_=rden_sp[:],
                )
                rden_bc = bp.tile([E, L], F32, tag="rden_bc")
                nc.sync.dma_start(
                    out=rden_bc[:],
                    in_=bass.AP(tensor=rdenrow_d, offset=b * L,
                                ap=[[0, E], [1, L]]),
                )
                stateT = bp.tile([E, L], BF16, tag="stateT")
                nc.vector.tensor_tensor(stateT[:], stu[:], rden_bc[:], MULT)

                # state_l init: second-orientation out_proj + rden scale
                state_l = bp.tile([128, NT * E], F32, tag="state_l")
                ps = psP.tile([128, 1024], F32, tag="ps")
                for lt in range(NT):
                    nc.tensor.matmul(
                        ps[:, lt * E:(lt + 1) * E],
                        o_un[:, lt * 128:(lt + 1) * 128], woutkt[:],
                        start=True, stop=True,
                    )
                nc.vector.tensor_tensor(
                    state_l[:].rearrange("p (j e) -> p j e", e=E),
                    ps[:].rearrange("p (j e) -> p j e", e=E),
                    rden_sp[:, :, None].to_broadcast([128, NT, E]),
                    MULT,
                )

                # ---- phase E: Euler steps ----
                t3 = bp.tile([128, NT * E], BF16, tag="t3")
                t3_v = t3[:].rearrange("p (n e) -> p n e", e=E)
                nc.vector.memset(t3_v[:, :, 0:1], 0.0)
                for t in range(F_LEN):
                    for g in range(8):   # 2-l-tile groups
                        ph = psP.tile([128, 1024], F32, tag="ps")
                        for u in range(2):
                            lt = 2 * g + u
                            nc.tensor.matmul(
                                ph[:, u * 512:u * 512 + 504],
                                stateT[:, lt * 128:(lt + 1) * 128], wall[:],
                                start=True, stop=True,
                            )
                        ph_v = ph[:].rearrange("p (u k) -> p u k", k=512)
                        ph_jd = ph_v[:, :, 0:504].rearrange(
                            "p u (j d) -> p u d j", d=D)
                        nc.vector.tensor_reduce(
                            t3_v[:, 2 * g:2 * g + 2, 1:E],
                            ph_jd, mybir.AxisListType.X, MULT,
                        )
                    # state_l += DT * t3   (64-wide tiles; col0 of t3 is zero)
                    slv = state_l[:]
                    nc.vector.scalar_tensor_tensor(
                        slv, t3[:], DT, slv, MULT, ADD,
                    )
                    # outbuf = state_l * s  (gpsimd)
                    outbuf = op_pool.tile([128, NT * E], F32, tag="outbuf")
                    nc.gpsimd.tensor_tensor(
                        outbuf[:].rearrange("p (n e) -> p n e", e=E),
                        state_l[:].rearrange("p (n e) -> p n e", e=E),
                        srep[:, None, :].to_broadcast([128, NT, E]),
                        MULT,
                    )
                    # vT via identity matmuls, then stateT += DT * vT
                    for g2 in range(2):
                        pvt = psP.tile([128, 1024], F32, tag="ps")
                        for j in range(8):
                            lt = g2 * 8 + j
                            nc.tensor.matmul(
                                pvt[0:E, j * 128:(j + 1) * 128],
                                t3[:, lt * E:(lt + 1) * E], ident[:],
                                start=True, stop=True,
                            )
                        stv = stateT[:, g2 * 1024:(g2 + 1) * 1024]
                        nc.vector.scalar_tensor_tensor(
                            stv, pvt[0:E, :], DT, stv, MULT, ADD,
                        )
                    # write out[b, :, t*63 : (t+1)*63]
                    nc.sync.dma_start(
                        out=bass.AP(tensor=out_e, offset=b * L * F_LEN * D + t * D,
                                    ap=[[F_LEN * D, 128], [128 * F_LEN * D, NT],
                                        [1, D]]),
                        in_=outbuf[:].rearrange("p (n e) -> p n e", e=E)[:, :, 1:E],
                    )

    _split_multiwaits(nc)
    return nc


_NC_CACHE = None


def _get_nc():
    global _NC_CACHE
    if _NC_CACHE is None:
        _NC_CACHE = _build_nc()
    return _NC_CACHE


def kernel(t, inputs, in_proj_w, in_proj_b, out_proj_w, out_proj_b,
           Wg, Mg, bg, sigma):
    inputs = np.asarray(inputs, np.float32)
    in_proj_w = np.asarray(in_proj_w, np.float32)
    in_proj_b = np.asarray(in_proj_b, np.float32)
    out_proj_w = np.asarray(out_proj_w, np.float32)
    out_proj_b = np.asarray(out_proj_b, np.float32)
    Wg = np.asarray(Wg, np.float32)
    Mg = np.asarray(Mg, np.float32)
    bg = np.asarray(bg, np.float32)
    sigma = np.asarray(sigma, np.float32)
    bf = mybir.dt.np(BF16)

    # ---- host-side weight prep ----
    s = sigma + EPS
    inv_s_aug = np.concatenate([[1.0], 1.0 / s]).astype(np.float32)
    Win_f = in_proj_w * inv_s_aug[None, :]
    scale = 1.0 / np.sqrt(np.float32(E))
    Wq = Win_f[0:E] * scale           # fold score scale into q projection
    Wk = Win_f[E:2 * E]
    Wv = Win_f[2 * E:3 * E]
    # (in_proj_b / out_proj_b are zeros in this model; asserted cheaply)
    assert np.all(in_proj_b == 0) and np.all(out_proj_b == 0)

    wqkt = np.concatenate([Wq, Wk], axis=0).T.astype(bf)       # [64, 128]
    wvt = Wv.T.astype(bf)                                      # [64, 64]
    Wout_aug = np.zeros((E, E + 1), np.float32)
    Wout_aug[0, E] = 1.0
    Wout_aug[1:, 0:E] = out_proj_w[1:, :]
    woutkt = Wout_aug.T.astype(bf)                             # [65, 64]

    Wgm = Wg * Mg
    Wall = np.zeros((E, D * W1), np.float32)
    for j in range(W1):
        Wall[:, j * D:(j + 1) * D] = Wgm[:, j, :].T
        Wall[0, j * D:(j + 1) * D] += bg[:, j]
    wall = Wall.astype(bf)

    masks = np.zeros((128, 4 * 512), np.float32)
    kv = np.arange(128)[:, None]
    q = np.arange(512)[None, :]
    for off in range(4):
        masks[:, off * 512:(off + 1) * 512] = (off * 128 + kv <= q)
    masks = masks.astype(bf)
    ident = np.eye(128).astype(bf)
    srep = np.zeros((128, E), np.float32)
    srep[:, 1:] = s[None, :]

    xt_all = np.ascontiguousarray(
        inputs.reshape(NCORES, BPC, L, E).transpose(0, 1, 3, 2)).astype(bf)

    in_maps = []
    for i in range(NCORES):
        in_maps.append({
            "xt": xt_all[i], "wqkt": wqkt, "wvt": wvt, "woutkt": woutkt,
            "wall": wall, "masks": masks, "ident": ident, "srep": srep,
        })

    nc = _get_nc()
    res = run_bass_kernel_spmd(nc, in_maps, core_ids=list(range(NCORES)))
    global LAST_RESULTS
    LAST_RESULTS = res
    out = np.concatenate([res.results[i]["out"] for i in range(NCORES)], axis=0)
    return np.ascontiguousarray(out.astype(np.float32))


LAST_RESULTS = None
